# revision 31
# baseline (speedup 1.0000x reference)
"""Distributed multi-head-attention kernel for 8 TRN2 NeuronCores.

Problem (hardcoded): B=4, T=S=1024, E=512, H=8, head_dim=64, fp32 I/O.
Sharding: core c handles batch b=c//2 and heads [4*(c%2), 4*(c%2)+4).
No collectives: each core produces a partial output projection
(contraction over its 256 ctx columns); the host sums the two partials
per batch and adds bo.

Compute dtype: bf16 on the TensorEngine (fp32 PSUM accumulation),
softmax in fp32 on ScalarE/VectorE.

v2 structure: heads processed as pairs (0,1) then (2,3). The even head
of a pair lives on SBUF partitions 0-63 of the qT/kT o-tile, the odd
head on 64-127, so adjacent score matmuls land in disjoint PE-array
row-strips and run concurrently (HW quadrant tiling). Context is
normalized into [128, T] head-pair tiles (odd head written with a
partition-shifted DVE mul) so the output projection runs K=128
full-width matmuls. Softmax denominators use skinny [1,128]-stationary
PE transposes instead of full-tile transposes.
"""

import numpy as np
import ml_dtypes

import concourse.bass as bass
import concourse.tile as tile
import concourse.mybir as mybir
from concourse.bass_utils import run_bass_kernel_spmd

BF16 = mybir.dt.bfloat16
F32 = mybir.dt.float32
NPBF16 = ml_dtypes.bfloat16

B, T, S, E = 4, 1024, 1024, 512
H, HD = 8, 64
N_CORES = 8
HPC = H // 2          # heads per core = 4
NP = HPC // 2         # head pairs per core = 2
OS = E // 2           # o-slice width per core = 256
KT = E // 128         # contraction k-tiles for projections = 4
TT = T // 128         # token tiles = 8
TC = T // 512         # 512-wide token chunks = 2

# ---------------------------------------------------------------------------
# Walrus in this container rejects instructions carrying more than a couple of
# sync waits. After Tile scheduling, split excess waits onto same-engine NOPs
# inserted immediately before the over-subscribed instruction.
# ---------------------------------------------------------------------------
_MAX_WAITS = 1
_split_ctr = [0]


def _split_sync_waits(nc, max_waits=_MAX_WAITS):
    for f in nc.m.functions:
        for bb in f.blocks:
            insts = bb.instructions
            if not any(i.sync_info and i.sync_info.on_wait
                       and len(i.sync_info.on_wait) > max_waits for i in insts):
                continue
            new = []
            for inst in insts:
                si = inst.sync_info
                if si is not None and si.on_wait and len(si.on_wait) > max_waits:
                    waits = list(si.on_wait)
                    extra, keep = waits[:-max_waits], waits[-max_waits:]
                    for j in range(0, len(extra), max_waits):
                        _split_ctr[0] += 1
                        nop = mybir.InstNoOp(
                            name=f"syncsplit-{_split_ctr[0]}", ins=[], outs=[])
                        nop.engine = inst.engine
                        nop.bass_nofuse = True
                        nop.text_hint = "syncsplit"
                        nop.sync_info = mybir.SyncInfo(
                            on_wait=extra[j:j + max_waits], on_update=[])
                        new.append(nop)
                    si.on_wait = keep
                new.append(inst)
            bb.instructions = new


# ---------------------------------------------------------------------------
# Kernel graph
# ---------------------------------------------------------------------------
def _drain_and_barrier_light(self, tick_clock, wait_clock):
    from concourse.vector_clock import ScopedClock
    nc = self.nc
    drain_inst = nc.sync.drain()
    wait_clock.add_sem_waits(
        drain_inst.ins, ScopedClock({None: tick_clock.global_clock}))
    nc.all_engine_barrier()
    assert self.sems is not None
    popped = nc._tile_sem_poison_stack.pop()
    assert popped is self._sem_poison
    nc.clear_and_free_semaphores(list(self.sems.allocated().values()))


tile.TileContext._drain_and_barrier = _drain_and_barrier_light


def build_nc():
    nc = bass.Bass()

    # p-major layouts: [p, k, n] flattened so DMAs are contiguous per partition
    xqT = nc.declare_dram_parameter("xqT", [128, KT * T], BF16, isOutput=False)
    xkT = nc.declare_dram_parameter("xkT", [128, KT * S], BF16, isOutput=False)
    xvT = nc.declare_dram_parameter("xvT", [128, KT * S], BF16, isOutput=False)
    wqT = nc.declare_dram_parameter("wqT", [128, KT * OS], BF16, isOutput=False)
    wkT = nc.declare_dram_parameter("wkT", [128, KT * OS], BF16, isOutput=False)
    wvT = nc.declare_dram_parameter("wvT", [128, KT * OS], BF16, isOutput=False)
    # head-pair slices of Wo^T: [128 (= 2 heads x 64 c), 512 (e)] each
    woPs = [nc.declare_dram_parameter(f"woP{p}", [128, E], BF16, isOutput=False)
            for p in range(NP)]
    bq_t = nc.declare_dram_parameter("bq_t", [128, 2], F32, isOutput=False)
    bk_t = nc.declare_dram_parameter("bk_t", [128, 2], F32, isOutput=False)
    bv_b = nc.declare_dram_parameter("bv_b", [128, OS], F32, isOutput=False)

    ident = nc.declare_dram_parameter("ident", [128, 128], F32, isOutput=False)
    indic = nc.declare_dram_parameter("indic", [TT, TT * HD], BF16,
                                      isOutput=False)
    out_ext = nc.declare_dram_parameter("out", [T, E], F32, isOutput=True)

    with tile.TileContext(nc) as tc:
        with (
            tc.tile_pool(name="inp", bufs=1) as inp,
            tc.tile_pool(name="wts", bufs=1) as wts,
            tc.tile_pool(name="act", bufs=1) as actp,
            tc.tile_pool(name="et", bufs=6) as etp,
            tc.tile_pool(name="rb", bufs=3) as rbp,
            tc.tile_pool(name="psum", bufs=1, space="PSUM") as psum,
        ):
            # ---- HAM warm-up: the PE clock-gate only reaches 2.4 GHz
            # after ~3.4us of continuous matmul activity, and the front's
            # DMA-stall gaps (~2.5-2.9us) are just under the re-throttle
            # window. Burn the otherwise-idle DMA-wait time on garbage
            # matmuls into a scratch PSUM bank so the projections (and
            # everything after) run at full clock.
            warm = inp.tile([128, 512], BF16, name="warm")
            nc.gpsimd.memset(warm[:], 0.0)
            wps = psum.tile([128, 512], F32, name="warmps", tag="sps",
                            bufs=2)
            for _ in range(12):
                nc.tensor.matmul(wps[:], warm[:, 0:128], warm[:],
                                 start=True, stop=True)

            # ---- input DMAs: three engine queues, ordered by need. The
            # wire is bandwidth-bound (~20us for 7MB), so the score-path
            # tensors (wq/wk/x q/k) go first, xv queues behind xk on the same
            # engine, and ident/wo trail everything.
            xq_sb = inp.tile([128, KT, T], BF16)
            xk_sb = inp.tile([128, KT, S], BF16)
            rrq = xqT.ap().rearrange("p (k t) -> p k t", k=KT)
            rrk = xkT.ap().rearrange("p (k t) -> p k t", k=KT)
            for k in range(KT):
                nc.sync.dma_start(xq_sb[:, k:k + 1, :], rrq[:, k:k + 1, :])
                nc.gpsimd.dma_start(xk_sb[:, k:k + 1, 0:512],
                                    rrk[:, k:k + 1, 0:512])
            for k in range(KT):
                nc.gpsimd.dma_start(xk_sb[:, k:k + 1, 512:1024],
                                    rrk[:, k:k + 1, 512:1024])

            wq_sb = wts.tile([128, KT, OS], BF16)
            nc.scalar.dma_start(wq_sb[:], wqT.ap().rearrange("p (k o) -> p k o", k=KT))
            wk_sb = wts.tile([128, KT, OS], BF16)
            nc.scalar.dma_start(wk_sb[:], wkT.ap().rearrange("p (k o) -> p k o", k=KT))
            bq_sb = wts.tile([128, 2], F32, name="bq")
            nc.scalar.dma_start(bq_sb[:], bq_t.ap())
            bk_sb = wts.tile([128, 2], F32, name="bk")
            nc.scalar.dma_start(bk_sb[:], bk_t.ap())

            xv_sb = inp.tile([128, KT, S], BF16)
            rrv = xvT.ap().rearrange("p (k t) -> p k t", k=KT)
            for k in range(KT):
                nc.gpsimd.dma_start(xv_sb[:, k:k + 1, :], rrv[:, k:k + 1, :])
            wv_sb = wts.tile([128, KT, OS], BF16)
            nc.sync.dma_start(wv_sb[:], wvT.ap().rearrange("p (k o) -> p k o", k=KT))
            bv_sb = wts.tile([128, OS], F32, name="bv")
            nc.sync.dma_start(bv_sb[:], bv_b.ap())
            id_sb = wts.tile([128, 128], F32, name="ident")
            nc.sync.dma_start(id_sb[:], ident.ap())
            ind_sb = wts.tile([TT, TT * HD], BF16, name="ind")
            nc.gpsimd.dma_start(ind_sb[:], indic.ap())
            wo_sbs = []
            for p in range(NP):
                wo_sb = wts.tile([128, E], BF16, name=f"wo{p}")
                nc.gpsimd.dma_start(wo_sb[:], woPs[p].ap())
                wo_sbs.append(wo_sb)

            # ---- persistent activations ------------------------------------
            # q^T, k^T: [o(128) x t] tiles; o-tile p holds heads 2p, 2p+1.
            qT_sb = [actp.tile([128, T], BF16, name=f"qT{p}") for p in range(NP)]
            kT_sb = [actp.tile([128, S], BF16, name=f"kT{p}") for p in range(NP)]
            v_aug = [actp.tile([128, HPC, HD + 1], BF16, name=f"vaug{st}")
                     for st in range(TT)]
            # normalized ctx for a head pair: even head on partitions 0-63,
            # odd head on 64-127
            ctx_pair = [actp.tile([128, T], BF16, name=f"ctx{p}")
                        for p in range(NP)]

            def qk_proj_quarter(p, which, tc_i):
                # One 512-wide chunk of the q^T or k^T projection for o-tile
                # p: 4 K-tile matmuls + a bias/cast copy.
                (src_sb, w_sb, b_sb, dst) = (
                    (xq_sb, wq_sb, bq_sb, qT_sb),
                    (xk_sb, wk_sb, bk_sb, kT_sb),
                )[which]
                ps = psum.tile([128, 512], F32, name="projq", tag="sps",
                               bufs=2)
                for k in range(KT):
                    nc.tensor.matmul(
                        ps[:],
                        w_sb[:, k, 128 * p:128 * (p + 1)],
                        src_sb[:, k, 512 * tc_i:512 * (tc_i + 1)],
                        start=(k == 0), stop=(k == KT - 1),
                    )
                nc.vector.tensor_scalar_add(
                    dst[p][:, 512 * tc_i:512 * (tc_i + 1)], ps[:],
                    b_sb[:, p:p + 1])

            def qk_proj(p):
                for which in range(2):
                    for tc_i in range(TC):
                        qk_proj_quarter(p, which, tc_i)

            def v_proj(st, tag="ctxps"):
                # v natural layout + 64 ones columns per head: the ctx matmul
                # then emits r replicated across output partitions 64-127 for
                # free (matmul cost is N cycles regardless of M), which IS
                # the across-partition broadcast the normalize needs.
                nc.gpsimd.memset(v_aug[st][:, :, HD:HD + 1], 1.0)
                ps = psum.tile([128, OS], F32, name="projv", tag=tag, bufs=2)
                for k in range(KT):
                    nc.tensor.matmul(
                        ps[:],
                        xv_sb[:, k, 128 * st:128 * (st + 1)],
                        wv_sb[:, k, :],
                        start=(k == 0), stop=(k == KT - 1),
                    )
                nc.vector.tensor_add(
                    v_aug[st][:, :, 0:HD],
                    ps.rearrange("p (h d) -> p h d", h=HPC),
                    bv_sb.rearrange("p (h d) -> p h d", h=HPC),
                )

            def scores_pair(p, st):
                # Score matmuls for both heads of the pair, interleaved so the
                # even head (PE rows 0-63) and odd head (rows 64-127) run
                # concurrently in disjoint row-strips.
                s_e = psum.tile([128, T], F32, name="sps_e", tag="sps", bufs=2)
                s_o = psum.tile([128, T], F32, name="sps_o", tag="sps", bufs=2)
                for tc_i in range(TC):
                    for half, s_ps in ((0, s_e), (1, s_o)):
                        po = HD * half
                        nc.tensor.matmul(
                            s_ps[:, 512 * tc_i:512 * (tc_i + 1)],
                            kT_sb[p][po:po + HD, 128 * st:128 * (st + 1)],
                            qT_sb[p][po:po + HD, 512 * tc_i:512 * (tc_i + 1)],
                            start=True, stop=True,
                        )
                return s_e, s_o

            def exp_tile(s_ps):
                et = etp.tile([128, T], BF16, name="et")
                nc.scalar.activation(
                    et[:], s_ps[:], mybir.ActivationFunctionType.Exp,
                    scale=float(1.0 / np.sqrt(HD)))
                return et

            def ctx_mm(c_ps, h, st, et):
                for tc_i in range(TC):
                    nc.tensor.matmul(
                        c_ps[:, 512 * tc_i:512 * (tc_i + 1)],
                        v_aug[st][:, h, :],
                        et[:, 512 * tc_i:512 * (tc_i + 1)],
                        start=(st == 0), stop=(st == TT - 1),
                    )

            def tail_front(p, half, c_ps, eng=None):
                # Copy the denominator row (PSUM partition 64 -> SBUF
                # partition 0); pair 0 also stages ctx rows so the PSUM
                # banks free before the next pair.
                rsb = rbp.tile([1, T], F32, name="rsb", bufs=4)
                if eng is nc.scalar:
                    nc.scalar.copy(rsb[0:1, :], c_ps[HD:HD + 1, :])
                else:
                    nc.vector.tensor_copy(rsb[0:1, :], c_ps[HD:HD + 1, :])
                cst = rbp.tile([HD, T], F32, name="cstage", bufs=4)
                if eng is nc.scalar:
                    nc.scalar.copy(cst[:], c_ps[0:HD, :])
                else:
                    nc.vector.tensor_copy(cst[:], c_ps[0:HD, :])
                return p, half, rsb, cst

            def make_tail(p, half, rsb, src_):
                # Skinny [1,128]-stationary PE transposes spread r over 128
                # lanes (DVE reciprocal is ~8 cycles/elem per lane, so the
                # free dim must be tiny), exact reciprocal on a [128, 8]
                # block, PE transpose back, then indicator-matmul broadcast
                # into a PSUM stripe for one [64, T] normalize mul.
                def tail():
                    tp = psum.tile([128, TT], F32, name="tps", tag="sps",
                                   bufs=2)
                    for c in range(TT):
                        nc.tensor.matmul(
                            tp[:, c:c + 1],
                            rsb[0:1, 128 * c:128 * (c + 1)],
                            id_sb[0:1, 0:1],
                            is_transpose=True, start=True, stop=True)
                    rinvT = rbp.tile([128, TT], F32, name="rinvT", bufs=4)
                    nc.vector.reciprocal(rinvT[:], tp[:, 0:TT])
                    tpb = psum.tile([TT, 128], F32, name="tpb", tag="sps",
                                    bufs=2)
                    nc.tensor.matmul(
                        tpb[:], rinvT[:], id_sb[:],
                        is_transpose=True, start=True, stop=True)
                    r8 = rbp.tile([TT, 128], BF16, name="r8", bufs=4)
                    nc.vector.tensor_copy(r8[:], tpb[:])
                    rb_ps = psum.tile([HD, T], F32, name="rbps", tag="sps",
                                      bufs=2)
                    for c in range(TT):
                        nc.tensor.matmul(
                            rb_ps[:, 128 * c:128 * (c + 1)],
                            ind_sb[:, HD * c:HD * (c + 1)],
                            r8[:], start=True, stop=True)
                    nc.vector.tensor_mul(
                        ctx_pair[p][HD * half:HD * (half + 1), :],
                        src_[0:HD, :], rb_ps[:])
                return tail

            # ---- phase 0: projections, scores(0), v --------------------------
            # qk_proj(1) needs no extra DMAs (same wq/wk/xq/xk), so it fills
            # the PE idle gaps while the front DMAs stream.
            qk_proj(0)
            s_cur = scores_pair(0, 0)
            et_e = exp_tile(s_cur[0])
            et_o = exp_tile(s_cur[1])
            s_n1 = scores_pair(0, 1)
            et_e1 = exp_tile(s_n1[0])
            et_o1 = exp_tile(s_n1[1])
            qk_proj(1)
            for st in range(TT):
                v_proj(st)

            # ---- head-pair pipeline ----------------------------------------
            # Per iteration (p, st): ctx for the even head (waits exp_e),
            # scores for st+1 (even/odd interleaved -> concurrent), ctx for
            # the odd head, plus one deferred chunk of pair-1's q/k
            # projection during pair 0. exp instructions are emitted in
            # (e, o) order per iteration to keep ScalarE saturated.
            pending_tails = []
            for p in range(NP):
                c_e = psum.tile([HD + 1, T], F32, name="ctx_e", tag="ctxps",
                                bufs=2)
                c_o = psum.tile([HD + 1, T], F32, name="ctx_o", tag="ctxps",
                                bufs=2)
                for st in range(TT):
                    ctx_mm(c_e, 2 * p, st, et_e)
                    if p == 0 and st == TT - 1:
                        a_e0 = tail_front(0, 0, c_e)
                    elif p == NP - 1 and st == TT - 1:
                        a_e1 = tail_front(p, 0, c_e)
                    # scores run two iterations ahead of ctx
                    if p == 0 and st + 2 < TT:
                        s_nxt = scores_pair(p, st + 2)
                    elif p == 0:
                        s_nxt = scores_pair(1, st + 2 - TT)
                    elif st + 2 < TT:
                        s_nxt = scores_pair(p, st + 2)
                    else:
                        s_nxt = None
                    ctx_mm(c_o, 2 * p + 1, st, et_o)
                    et_e, et_o = et_e1, et_o1
                    if s_nxt is not None:
                        et_e1 = exp_tile(s_nxt[0])
                        et_o1 = exp_tile(s_nxt[1])
                    # pair-0 normalize tails during pair 1
                    if p > 0 and pending_tails and st in (2, 4):
                        pending_tails.pop(0)()
                if p + 1 < NP:
                    a_o = tail_front(p, 1, c_o)
                    pending_tails.append(make_tail(*a_e0))
                    pending_tails.append(make_tail(*a_o))

            # ---- last pair's tails; outproj pair-0 matmuls interleave ------
            o_pss = {}

            def outproj_first(g):
                o_ps = psum.tile([128, E], F32, name="ops",
                                 tag=("sps" if g % 2 == 0 else "ctxps"),
                                 bufs=2)
                o_pss[g] = o_ps
                nc.tensor.matmul(
                    o_ps[:], ctx_pair[0][:, 128 * g:128 * (g + 1)],
                    wo_sbs[0][:], start=True, stop=False)

            a_o1 = tail_front(NP - 1, 1, c_o, eng=nc.scalar)
            outproj_first(0)
            # both endgame tails with stages interleaved so PE work of one
            # overlaps DVE work of the other
            tps, rinvTs, tpbs, r8s, rbps = [], [], [], [], []
            for (p_, half_, rsb_, cst_) in (a_e1, a_o1):
                tp = psum.tile([128, TT], F32, name="tps", tag="sps", bufs=2)
                for c in range(TT):
                    nc.tensor.matmul(
                        tp[:, c:c + 1], rsb_[0:1, 128 * c:128 * (c + 1)],
                        id_sb[0:1, 0:1], is_transpose=True, start=True,
                        stop=True)
                tps.append(tp)
            for tp in tps:
                rinvT = rbp.tile([128, TT], F32, name="rinvT", bufs=4)
                nc.vector.reciprocal(rinvT[:], tp[:, 0:TT])
                rinvTs.append(rinvT)
            outproj_first(1)
            for rinvT in rinvTs:
                tpb = psum.tile([TT, 128], F32, name="tpb", tag="sps", bufs=2)
                nc.tensor.matmul(tpb[:], rinvT[:], id_sb[:],
                                 is_transpose=True, start=True, stop=True)
                tpbs.append(tpb)
            for i, tpb in enumerate(tpbs):
                r8 = rbp.tile([TT, 128], BF16, name="r8", bufs=4)
                (nc.vector if i == 0 else nc.scalar).copy(r8[:], tpb[:]) \
                    if i else nc.vector.tensor_copy(r8[:], tpb[:])
                r8s.append(r8)
            for r8 in r8s:
                rb_ps = psum.tile([HD, T], F32, name="rbps", tag="sps",
                                  bufs=2)
                for c in range(TT):
                    nc.tensor.matmul(
                        rb_ps[:, 128 * c:128 * (c + 1)],
                        ind_sb[:, HD * c:HD * (c + 1)],
                        r8[:], start=True, stop=True)
                rbps.append(rb_ps)
            for (p_, half_, rsb_, cst_), rb_ps in zip((a_e1, a_o1), rbps):
                nc.vector.tensor_mul(
                    ctx_pair[p_][HD * half_:HD * (half_ + 1), :],
                    cst_[0:HD, :], rb_ps[:])
            for g in range(2, 4):
                outproj_first(g)
            for g in range(TT):
                o_ps = o_pss.pop(g)
                nc.tensor.matmul(
                    o_ps[:], ctx_pair[1][:, 128 * g:128 * (g + 1)],
                    wo_sbs[1][:], start=False, stop=True)
                if g + 4 < TT:
                    outproj_first(g + 4)
                o_sb = rbp.tile([128, E], F32, name="osb", bufs=4)
                if g % 2 == 0:
                    nc.scalar.copy(o_sb[:], o_ps[:])
                else:
                    nc.vector.tensor_copy(o_sb[:], o_ps[:])
                (nc.sync if g % 2 == 0 else nc.gpsimd).dma_start(
                    out_ext.ap().rearrange("(g pp) e -> pp g e", pp=128)
                    [:, g:g + 1, :],
                    o_sb.rearrange("p (g e) -> p g e", g=1))

    _split_sync_waits(nc)
    return nc


_NC = None


def _get_nc():
    global _NC
    if _NC is None:
        _NC = build_nc()
    return _NC


# ---------------------------------------------------------------------------
# Host-side sharding / unsharding
# ---------------------------------------------------------------------------
def make_in_maps(queries, keys, values, Wq, bq, Wk, bk, Wv, bv, Wo):
    in_maps = []
    for c in range(N_CORES):
        b, hh = divmod(c, 2)
        osl = slice(OS * hh, OS * (hh + 1))
        bq_s = np.zeros((128, 2), np.float32)
        bq_s[:, 0] = bq[osl][0:128]
        bq_s[:, 1] = bq[osl][128:256]
        bk_s = np.zeros((128, 2), np.float32)
        bk_s[:, 0] = bk[osl][0:128]
        bk_s[:, 1] = bk[osl][128:256]

        def pmaj(a):
            # [E, N] -> [128, KT*N], k-tiles along the free axis
            e, n = a.shape
            return np.ascontiguousarray(
                a.reshape(KT, 128, n).transpose(1, 0, 2).reshape(128, KT * n))

        m = {
            "xqT": pmaj(queries[b].T).astype(NPBF16),
            "xkT": pmaj(keys[b].T).astype(NPBF16),
            "xvT": pmaj(values[b].T).astype(NPBF16),
            "wqT": pmaj(Wq[osl, :].T).astype(NPBF16),
            "wkT": pmaj(Wk[osl, :].T).astype(NPBF16),
            "wvT": pmaj(Wv[osl, :].T).astype(NPBF16),
            "bq_t": bq_s,
            "bk_t": bk_s,
            "bv_b": np.broadcast_to(
                bv[osl][None, :], (128, OS)).astype(np.float32).copy(),
            "ident": np.eye(128, dtype=np.float32),
            "indic": np.repeat(np.eye(TT), HD, axis=1).astype(NPBF16),
        }
        for p in range(NP):
            cs = slice(OS * hh + 128 * p, OS * hh + 128 * (p + 1))
            m[f"woP{p}"] = np.ascontiguousarray(Wo[:, cs].T).astype(NPBF16)
        in_maps.append(m)
    return in_maps


def run_device(in_maps, trace=False):
    nc = _get_nc()
    return run_bass_kernel_spmd(
        nc, in_maps, core_ids=list(range(N_CORES)), trace=trace)


def _numpy_reference(queries, keys, values, Wq, bq, Wk, bk, Wv, bv, Wo, bo,
                     q_padding_mask, key_padding_mask, attn_mask):
    q = queries @ Wq.T + bq
    k = keys @ Wk.T + bk
    v = values @ Wv.T + bv

    def split(x):
        b, l, e = x.shape
        return x.reshape(b, l, H, HD).transpose(0, 2, 1, 3)

    q, k, v = split(q), split(k), split(v)
    scores = np.einsum('bhtd,bhsd->bhts', q, k) / np.sqrt(HD)
    scores = np.where(key_padding_mask[:, None, None, :], -np.inf, scores)
    scores = np.where(~attn_mask[None, None, :, :], -np.inf, scores)
    scores = scores - scores.max(axis=-1, keepdims=True)
    w = np.exp(scores)
    w = w / w.sum(axis=-1, keepdims=True)
    w = np.where(q_padding_mask[:, None, :, None], 0.0, w)
    ctx = np.einsum('bhts,bhsd->bhtd', w, v)
    ctx = ctx.transpose(0, 2, 1, 3).reshape(queries.shape[0], -1, E)
    return (ctx @ Wo.T + bo).astype(np.float32)


def kernel(queries, keys, values, Wq, bq, Wk, bk, Wv, bv, Wo, bo,
           q_padding_mask, key_padding_mask, attn_mask):
    queries = np.asarray(queries, dtype=np.float32)
    keys = np.asarray(keys, dtype=np.float32)
    values = np.asarray(values, dtype=np.float32)
    Wq, bq = np.asarray(Wq, np.float32), np.asarray(bq, np.float32)
    Wk, bk = np.asarray(Wk, np.float32), np.asarray(bk, np.float32)
    Wv, bv = np.asarray(Wv, np.float32), np.asarray(bv, np.float32)
    Wo, bo = np.asarray(Wo, np.float32), np.asarray(bo, np.float32)
    q_padding_mask = np.asarray(q_padding_mask)
    key_padding_mask = np.asarray(key_padding_mask)
    attn_mask = np.asarray(attn_mask)

    # The device kernel skips masking (and softmax max-subtraction, valid for
    # this problem's bounded score range). Masks are all-trivial per the
    # problem spec; fall back to a host reference if they ever are not.
    if q_padding_mask.any() or key_padding_mask.any() or not attn_mask.all():
        return _numpy_reference(
            queries, keys, values, Wq, bq, Wk, bk, Wv, bv, Wo, bo,
            q_padding_mask, key_padding_mask, attn_mask)

    in_maps = make_in_maps(queries, keys, values, Wq, bq, Wk, bk, Wv, bv, Wo)
    res = run_device(in_maps, trace=False)
    out = np.empty((B, T, E), np.float32)
    for b in range(B):
        out[b] = (res.results[2 * b]["out"] + res.results[2 * b + 1]["out"]
                  + bo[None, :])
    return out


# revision 32
# speedup vs baseline: 1.1522x; 1.1522x over previous
"""Distributed multi-head-attention kernel for 8 TRN2 NeuronCores.

Problem (hardcoded): B=4, T=S=1024, E=512, H=8, head_dim=64, fp32 I/O.
Sharding: core c handles batch b=c//2 and heads [4*(c%2), 4*(c%2)+4).
No collectives: each core produces a partial output projection
(contraction over its 256 ctx columns); the host sums the two partials
per batch and adds bo.

Compute dtype: bf16 on the TensorEngine (fp32 PSUM accumulation),
softmax in fp32 on ScalarE/VectorE.

v2 structure: heads processed as pairs (0,1) then (2,3). The even head
of a pair lives on SBUF partitions 0-63 of the qT/kT o-tile, the odd
head on 64-127, so adjacent score matmuls land in disjoint PE-array
row-strips and run concurrently (HW quadrant tiling). Context is
normalized into [128, T] head-pair tiles (odd head written with a
partition-shifted DVE mul) so the output projection runs K=128
full-width matmuls. Softmax denominators use skinny [1,128]-stationary
PE transposes instead of full-tile transposes.
"""

import numpy as np
import ml_dtypes

import concourse.bass as bass
import concourse.tile as tile
import concourse.mybir as mybir
from concourse.bass_utils import run_bass_kernel_spmd

BF16 = mybir.dt.bfloat16
F32 = mybir.dt.float32
NPBF16 = ml_dtypes.bfloat16

B, T, S, E = 4, 1024, 1024, 512
H, HD = 8, 64
N_CORES = 8
HPC = H // 2          # heads per core = 4
NP = HPC // 2         # head pairs per core = 2
OS = E // 2           # o-slice width per core = 256
KT = E // 128         # contraction k-tiles for projections = 4
TT = T // 128         # token tiles = 8
TC = T // 512         # 512-wide token chunks = 2

# ---------------------------------------------------------------------------
# Walrus in this container rejects instructions carrying more than a couple of
# sync waits. After Tile scheduling, split excess waits onto same-engine NOPs
# inserted immediately before the over-subscribed instruction.
# ---------------------------------------------------------------------------
_MAX_WAITS = 1
_split_ctr = [0]


def _split_sync_waits(nc, max_waits=_MAX_WAITS):
    for f in nc.m.functions:
        for bb in f.blocks:
            insts = bb.instructions
            if not any(i.sync_info and i.sync_info.on_wait
                       and len(i.sync_info.on_wait) > max_waits for i in insts):
                continue
            new = []
            for inst in insts:
                si = inst.sync_info
                if si is not None and si.on_wait and len(si.on_wait) > max_waits:
                    waits = list(si.on_wait)
                    extra, keep = waits[:-max_waits], waits[-max_waits:]
                    for j in range(0, len(extra), max_waits):
                        _split_ctr[0] += 1
                        nop = mybir.InstNoOp(
                            name=f"syncsplit-{_split_ctr[0]}", ins=[], outs=[])
                        nop.engine = inst.engine
                        nop.bass_nofuse = True
                        nop.text_hint = "syncsplit"
                        nop.sync_info = mybir.SyncInfo(
                            on_wait=extra[j:j + max_waits], on_update=[])
                        new.append(nop)
                    si.on_wait = keep
                new.append(inst)
            bb.instructions = new


# ---------------------------------------------------------------------------
# Kernel graph
# ---------------------------------------------------------------------------
def _drain_and_barrier_light(self, tick_clock, wait_clock):
    from concourse.vector_clock import ScopedClock
    nc = self.nc
    drain_inst = nc.sync.drain()
    wait_clock.add_sem_waits(
        drain_inst.ins, ScopedClock({None: tick_clock.global_clock}))
    nc.all_engine_barrier()
    assert self.sems is not None
    popped = nc._tile_sem_poison_stack.pop()
    assert popped is self._sem_poison
    nc.clear_and_free_semaphores(list(self.sems.allocated().values()))


tile.TileContext._drain_and_barrier = _drain_and_barrier_light


def build_nc():
    nc = bass.Bass()

    # p-major layouts: [p, k, n] flattened so DMAs are contiguous per partition
    xqT = nc.declare_dram_parameter("xqT", [128, KT * T], BF16, isOutput=False)
    xkT = nc.declare_dram_parameter("xkT", [128, KT * S], BF16, isOutput=False)
    xvT = nc.declare_dram_parameter("xvT", [128, KT * S], BF16, isOutput=False)
    wqT = nc.declare_dram_parameter("wqT", [128, KT * OS], BF16, isOutput=False)
    wkT = nc.declare_dram_parameter("wkT", [128, KT * OS], BF16, isOutput=False)
    wvT = nc.declare_dram_parameter("wvT", [128, KT * OS], BF16, isOutput=False)
    # head-pair slices of Wo^T: [128 (= 2 heads x 64 c), 512 (e)] each
    woPs = [nc.declare_dram_parameter(f"woP{p}", [128, E], BF16, isOutput=False)
            for p in range(NP)]
    bq_t = nc.declare_dram_parameter("bq_t", [128, 2], F32, isOutput=False)
    bk_t = nc.declare_dram_parameter("bk_t", [128, 2], F32, isOutput=False)
    bv_b = nc.declare_dram_parameter("bv_b", [128, OS], F32, isOutput=False)

    ident = nc.declare_dram_parameter("ident", [128, 128], F32, isOutput=False)
    indic = nc.declare_dram_parameter("indic", [TT, TT * HD], BF16,
                                      isOutput=False)
    out_ext = nc.declare_dram_parameter("out", [T, E], F32, isOutput=True)

    with tile.TileContext(nc) as tc:
        with (
            tc.tile_pool(name="inp", bufs=1) as inp,
            tc.tile_pool(name="wts", bufs=1) as wts,
            tc.tile_pool(name="act", bufs=1) as actp,
            tc.tile_pool(name="et", bufs=6) as etp,
            tc.tile_pool(name="rb", bufs=3) as rbp,
            tc.tile_pool(name="psum", bufs=1, space="PSUM") as psum,
        ):
            # ---- input DMAs: three engine queues, ordered by need. The
            # wire is bandwidth-bound (~20us for 7MB), so the score-path
            # tensors (wq/wk/x q/k) go first, xv queues behind xk on the same
            # engine, and ident/wo trail everything.
            xq_sb = inp.tile([128, KT, T], BF16)
            xk_sb = inp.tile([128, KT, S], BF16)
            rrq = xqT.ap().rearrange("p (k t) -> p k t", k=KT)
            rrk = xkT.ap().rearrange("p (k t) -> p k t", k=KT)
            for k in range(KT):
                nc.sync.dma_start(xq_sb[:, k:k + 1, :], rrq[:, k:k + 1, :])
                nc.gpsimd.dma_start(xk_sb[:, k:k + 1, 0:512],
                                    rrk[:, k:k + 1, 0:512])
            for k in range(KT):
                nc.gpsimd.dma_start(xk_sb[:, k:k + 1, 512:1024],
                                    rrk[:, k:k + 1, 512:1024])

            wq_sb = wts.tile([128, KT, OS], BF16)
            nc.scalar.dma_start(wq_sb[:], wqT.ap().rearrange("p (k o) -> p k o", k=KT))
            wk_sb = wts.tile([128, KT, OS], BF16)
            nc.scalar.dma_start(wk_sb[:], wkT.ap().rearrange("p (k o) -> p k o", k=KT))
            bq_sb = wts.tile([128, 2], F32, name="bq")
            nc.scalar.dma_start(bq_sb[:], bq_t.ap())
            bk_sb = wts.tile([128, 2], F32, name="bk")
            nc.scalar.dma_start(bk_sb[:], bk_t.ap())

            xv_sb = inp.tile([128, KT, S], BF16)
            rrv = xvT.ap().rearrange("p (k t) -> p k t", k=KT)
            for k in range(KT):
                nc.gpsimd.dma_start(xv_sb[:, k:k + 1, :], rrv[:, k:k + 1, :])
            wv_sb = wts.tile([128, KT, OS], BF16)
            nc.sync.dma_start(wv_sb[:], wvT.ap().rearrange("p (k o) -> p k o", k=KT))
            bv_sb = wts.tile([128, OS], F32, name="bv")
            nc.sync.dma_start(bv_sb[:], bv_b.ap())
            id_sb = wts.tile([128, 128], F32, name="ident")
            nc.sync.dma_start(id_sb[:], ident.ap())
            ind_sb = wts.tile([TT, TT * HD], BF16, name="ind")
            nc.gpsimd.dma_start(ind_sb[:], indic.ap())
            wo_sbs = []
            for p in range(NP):
                wo_sb = wts.tile([128, E], BF16, name=f"wo{p}")
                nc.gpsimd.dma_start(wo_sb[:], woPs[p].ap())
                wo_sbs.append(wo_sb)

            # ---- persistent activations ------------------------------------
            # q^T, k^T: [o(128) x t] tiles; o-tile p holds heads 2p, 2p+1.
            qT_sb = [actp.tile([128, T], BF16, name=f"qT{p}") for p in range(NP)]
            kT_sb = [actp.tile([128, S], BF16, name=f"kT{p}") for p in range(NP)]
            v_aug = [actp.tile([128, HPC, HD + 1], BF16, name=f"vaug{st}")
                     for st in range(TT)]
            # normalized ctx for a head pair: even head on partitions 0-63,
            # odd head on 64-127
            ctx_pair = [actp.tile([128, T], BF16, name=f"ctx{p}")
                        for p in range(NP)]

            def qk_proj_quarter(p, which, tc_i):
                # One 512-wide chunk of the q^T or k^T projection for o-tile
                # p: 4 K-tile matmuls + a bias/cast copy.
                (src_sb, w_sb, b_sb, dst) = (
                    (xq_sb, wq_sb, bq_sb, qT_sb),
                    (xk_sb, wk_sb, bk_sb, kT_sb),
                )[which]
                ps = psum.tile([128, 512], F32, name="projq", tag="sps",
                               bufs=2)
                for k in range(KT):
                    nc.tensor.matmul(
                        ps[:],
                        w_sb[:, k, 128 * p:128 * (p + 1)],
                        src_sb[:, k, 512 * tc_i:512 * (tc_i + 1)],
                        start=(k == 0), stop=(k == KT - 1),
                    )
                nc.vector.tensor_scalar_add(
                    dst[p][:, 512 * tc_i:512 * (tc_i + 1)], ps[:],
                    b_sb[:, p:p + 1])

            def qk_proj(p):
                for which in range(2):
                    for tc_i in range(TC):
                        qk_proj_quarter(p, which, tc_i)

            def v_proj(st, tag="ctxps"):
                # v natural layout + 64 ones columns per head: the ctx matmul
                # then emits r replicated across output partitions 64-127 for
                # free (matmul cost is N cycles regardless of M), which IS
                # the across-partition broadcast the normalize needs.
                nc.gpsimd.memset(v_aug[st][:, :, HD:HD + 1], 1.0)
                ps = psum.tile([128, OS], F32, name="projv", tag=tag, bufs=2)
                for k in range(KT):
                    nc.tensor.matmul(
                        ps[:],
                        xv_sb[:, k, 128 * st:128 * (st + 1)],
                        wv_sb[:, k, :],
                        start=(k == 0), stop=(k == KT - 1),
                    )
                nc.vector.tensor_add(
                    v_aug[st][:, :, 0:HD],
                    ps.rearrange("p (h d) -> p h d", h=HPC),
                    bv_sb.rearrange("p (h d) -> p h d", h=HPC),
                )

            def scores_pair(p, st):
                # Score matmuls for both heads of the pair, interleaved so the
                # even head (PE rows 0-63) and odd head (rows 64-127) run
                # concurrently in disjoint row-strips.
                s_e = psum.tile([128, T], F32, name="sps_e", tag="sps", bufs=2)
                s_o = psum.tile([128, T], F32, name="sps_o", tag="sps", bufs=2)
                for tc_i in range(TC):
                    for half, s_ps in ((0, s_e), (1, s_o)):
                        po = HD * half
                        nc.tensor.matmul(
                            s_ps[:, 512 * tc_i:512 * (tc_i + 1)],
                            kT_sb[p][po:po + HD, 128 * st:128 * (st + 1)],
                            qT_sb[p][po:po + HD, 512 * tc_i:512 * (tc_i + 1)],
                            start=True, stop=True,
                        )
                return s_e, s_o

            def exp_tile(s_ps):
                et = etp.tile([128, T], BF16, name="et")
                nc.scalar.activation(
                    et[:], s_ps[:], mybir.ActivationFunctionType.Exp,
                    scale=float(1.0 / np.sqrt(HD)))
                return et

            def ctx_mm(c_ps, h, st, et):
                for tc_i in range(TC):
                    nc.tensor.matmul(
                        c_ps[:, 512 * tc_i:512 * (tc_i + 1)],
                        v_aug[st][:, h, :],
                        et[:, 512 * tc_i:512 * (tc_i + 1)],
                        start=(st == 0), stop=(st == TT - 1),
                    )

            def tail_front(p, half, c_ps, eng=None):
                # Copy the denominator row (PSUM partition 64 -> SBUF
                # partition 0); pair 0 also stages ctx rows so the PSUM
                # banks free before the next pair.
                rsb = rbp.tile([1, T], F32, name="rsb", bufs=4)
                if eng is nc.scalar:
                    nc.scalar.copy(rsb[0:1, :], c_ps[HD:HD + 1, :])
                else:
                    nc.vector.tensor_copy(rsb[0:1, :], c_ps[HD:HD + 1, :])
                cst = rbp.tile([HD, T], F32, name="cstage", bufs=4)
                if eng is nc.scalar:
                    nc.scalar.copy(cst[:], c_ps[0:HD, :])
                else:
                    nc.vector.tensor_copy(cst[:], c_ps[0:HD, :])
                return p, half, rsb, cst

            def make_tail(p, half, rsb, src_):
                # Skinny [1,128]-stationary PE transposes spread r over 128
                # lanes (DVE reciprocal is ~8 cycles/elem per lane, so the
                # free dim must be tiny), exact reciprocal on a [128, 8]
                # block, PE transpose back, then indicator-matmul broadcast
                # into a PSUM stripe for one [64, T] normalize mul.
                def tail():
                    tp = psum.tile([128, TT], F32, name="tps", tag="sps",
                                   bufs=2)
                    for c in range(TT):
                        nc.tensor.matmul(
                            tp[:, c:c + 1],
                            rsb[0:1, 128 * c:128 * (c + 1)],
                            id_sb[0:1, 0:1],
                            is_transpose=True, start=True, stop=True)
                    rinvT = rbp.tile([128, TT], F32, name="rinvT", bufs=4)
                    nc.vector.reciprocal(rinvT[:], tp[:, 0:TT])
                    tpb = psum.tile([TT, 128], F32, name="tpb", tag="sps",
                                    bufs=2)
                    nc.tensor.matmul(
                        tpb[:], rinvT[:], id_sb[:],
                        is_transpose=True, start=True, stop=True)
                    r8 = rbp.tile([TT, 128], BF16, name="r8", bufs=4)
                    nc.vector.tensor_copy(r8[:], tpb[:])
                    rb_ps = psum.tile([HD, T], F32, name="rbps", tag="sps",
                                      bufs=2)
                    for c in range(TT):
                        nc.tensor.matmul(
                            rb_ps[:, 128 * c:128 * (c + 1)],
                            ind_sb[:, HD * c:HD * (c + 1)],
                            r8[:], start=True, stop=True)
                    nc.vector.tensor_mul(
                        ctx_pair[p][HD * half:HD * (half + 1), :],
                        src_[0:HD, :], rb_ps[:])
                return tail

            # ---- phase 0: projections, scores(0), v --------------------------
            # qk_proj(1) needs no extra DMAs (same wq/wk/xq/xk), so it fills
            # the PE idle gaps while the front DMAs stream.
            qk_proj(0)
            s_cur = scores_pair(0, 0)
            et_e = exp_tile(s_cur[0])
            et_o = exp_tile(s_cur[1])
            s_n1 = scores_pair(0, 1)
            et_e1 = exp_tile(s_n1[0])
            et_o1 = exp_tile(s_n1[1])
            qk_proj(1)
            for st in range(TT):
                v_proj(st)

            # ---- head-pair pipeline ----------------------------------------
            # Per iteration (p, st): ctx for the even head (waits exp_e),
            # scores for st+1 (even/odd interleaved -> concurrent), ctx for
            # the odd head, plus one deferred chunk of pair-1's q/k
            # projection during pair 0. exp instructions are emitted in
            # (e, o) order per iteration to keep ScalarE saturated.
            pending_tails = []
            for p in range(NP):
                c_e = psum.tile([HD + 1, T], F32, name="ctx_e", tag="ctxps",
                                bufs=2)
                c_o = psum.tile([HD + 1, T], F32, name="ctx_o", tag="ctxps",
                                bufs=2)
                for st in range(TT):
                    ctx_mm(c_e, 2 * p, st, et_e)
                    if p == 0 and st == TT - 1:
                        a_e0 = tail_front(0, 0, c_e)
                    elif p == NP - 1 and st == TT - 1:
                        a_e1 = tail_front(p, 0, c_e)
                    # scores run two iterations ahead of ctx
                    if p == 0 and st + 2 < TT:
                        s_nxt = scores_pair(p, st + 2)
                    elif p == 0:
                        s_nxt = scores_pair(1, st + 2 - TT)
                    elif st + 2 < TT:
                        s_nxt = scores_pair(p, st + 2)
                    else:
                        s_nxt = None
                    ctx_mm(c_o, 2 * p + 1, st, et_o)
                    et_e, et_o = et_e1, et_o1
                    if s_nxt is not None:
                        et_e1 = exp_tile(s_nxt[0])
                        et_o1 = exp_tile(s_nxt[1])
                    # pair-0 normalize tails during pair 1
                    if p > 0 and pending_tails and st in (2, 4):
                        pending_tails.pop(0)()
                if p + 1 < NP:
                    a_o = tail_front(p, 1, c_o)
                    pending_tails.append(make_tail(*a_e0))
                    pending_tails.append(make_tail(*a_o))

            # ---- last pair's tails; outproj pair-0 matmuls interleave ------
            o_pss = {}

            def outproj_first(g):
                o_ps = psum.tile([128, E], F32, name="ops",
                                 tag=("sps" if g % 2 == 0 else "ctxps"),
                                 bufs=2)
                o_pss[g] = o_ps
                nc.tensor.matmul(
                    o_ps[:], ctx_pair[0][:, 128 * g:128 * (g + 1)],
                    wo_sbs[0][:], start=True, stop=False)

            a_o1 = tail_front(NP - 1, 1, c_o, eng=nc.scalar)
            outproj_first(0)
            # both endgame tails with stages interleaved so PE work of one
            # overlaps DVE work of the other
            tps, rinvTs, tpbs, r8s, rbps = [], [], [], [], []
            for (p_, half_, rsb_, cst_) in (a_e1, a_o1):
                tp = psum.tile([128, TT], F32, name="tps", tag="sps", bufs=2)
                for c in range(TT):
                    nc.tensor.matmul(
                        tp[:, c:c + 1], rsb_[0:1, 128 * c:128 * (c + 1)],
                        id_sb[0:1, 0:1], is_transpose=True, start=True,
                        stop=True)
                tps.append(tp)
            for tp in tps:
                rinvT = rbp.tile([128, TT], F32, name="rinvT", bufs=4)
                nc.vector.reciprocal(rinvT[:], tp[:, 0:TT])
                rinvTs.append(rinvT)
            outproj_first(1)
            for rinvT in rinvTs:
                tpb = psum.tile([TT, 128], F32, name="tpb", tag="sps", bufs=2)
                nc.tensor.matmul(tpb[:], rinvT[:], id_sb[:],
                                 is_transpose=True, start=True, stop=True)
                tpbs.append(tpb)
            for i, tpb in enumerate(tpbs):
                r8 = rbp.tile([TT, 128], BF16, name="r8", bufs=4)
                (nc.vector if i == 0 else nc.scalar).copy(r8[:], tpb[:]) \
                    if i else nc.vector.tensor_copy(r8[:], tpb[:])
                r8s.append(r8)
            for r8 in r8s:
                rb_ps = psum.tile([HD, T], F32, name="rbps", tag="sps",
                                  bufs=2)
                for c in range(TT):
                    nc.tensor.matmul(
                        rb_ps[:, 128 * c:128 * (c + 1)],
                        ind_sb[:, HD * c:HD * (c + 1)],
                        r8[:], start=True, stop=True)
                rbps.append(rb_ps)
            for (p_, half_, rsb_, cst_), rb_ps in zip((a_e1, a_o1), rbps):
                nc.vector.tensor_mul(
                    ctx_pair[p_][HD * half_:HD * (half_ + 1), :],
                    cst_[0:HD, :], rb_ps[:])
            for g in range(2, 4):
                outproj_first(g)
            for g in range(TT):
                o_ps = o_pss.pop(g)
                nc.tensor.matmul(
                    o_ps[:], ctx_pair[1][:, 128 * g:128 * (g + 1)],
                    wo_sbs[1][:], start=False, stop=True)
                if g + 4 < TT:
                    outproj_first(g + 4)
                o_sb = rbp.tile([128, E], F32, name="osb", bufs=4)
                if g % 2 == 0:
                    nc.scalar.copy(o_sb[:], o_ps[:])
                else:
                    nc.vector.tensor_copy(o_sb[:], o_ps[:])
                (nc.sync if g % 2 == 0 else nc.gpsimd).dma_start(
                    out_ext.ap().rearrange("(g pp) e -> pp g e", pp=128)
                    [:, g:g + 1, :],
                    o_sb.rearrange("p (g e) -> p g e", g=1))

    _split_sync_waits(nc)
    return nc


_NC = None


def _get_nc():
    global _NC
    if _NC is None:
        _NC = build_nc()
    return _NC


# ---------------------------------------------------------------------------
# Host-side sharding / unsharding
# ---------------------------------------------------------------------------
def make_in_maps(queries, keys, values, Wq, bq, Wk, bk, Wv, bv, Wo):
    in_maps = []
    for c in range(N_CORES):
        b, hh = divmod(c, 2)
        osl = slice(OS * hh, OS * (hh + 1))
        bq_s = np.zeros((128, 2), np.float32)
        bq_s[:, 0] = bq[osl][0:128]
        bq_s[:, 1] = bq[osl][128:256]
        bk_s = np.zeros((128, 2), np.float32)
        bk_s[:, 0] = bk[osl][0:128]
        bk_s[:, 1] = bk[osl][128:256]

        def pmaj(a):
            # [E, N] -> [128, KT*N], k-tiles along the free axis
            e, n = a.shape
            return np.ascontiguousarray(
                a.reshape(KT, 128, n).transpose(1, 0, 2).reshape(128, KT * n))

        m = {
            "xqT": pmaj(queries[b].T).astype(NPBF16),
            "xkT": pmaj(keys[b].T).astype(NPBF16),
            "xvT": pmaj(values[b].T).astype(NPBF16),
            "wqT": pmaj(Wq[osl, :].T).astype(NPBF16),
            "wkT": pmaj(Wk[osl, :].T).astype(NPBF16),
            "wvT": pmaj(Wv[osl, :].T).astype(NPBF16),
            "bq_t": bq_s,
            "bk_t": bk_s,
            "bv_b": np.broadcast_to(
                bv[osl][None, :], (128, OS)).astype(np.float32).copy(),
            "ident": np.eye(128, dtype=np.float32),
            "indic": np.repeat(np.eye(TT), HD, axis=1).astype(NPBF16),
        }
        for p in range(NP):
            cs = slice(OS * hh + 128 * p, OS * hh + 128 * (p + 1))
            m[f"woP{p}"] = np.ascontiguousarray(Wo[:, cs].T).astype(NPBF16)
        in_maps.append(m)
    return in_maps


def run_device(in_maps, trace=False):
    nc = _get_nc()
    return run_bass_kernel_spmd(
        nc, in_maps, core_ids=list(range(N_CORES)), trace=trace)


def _numpy_reference(queries, keys, values, Wq, bq, Wk, bk, Wv, bv, Wo, bo,
                     q_padding_mask, key_padding_mask, attn_mask):
    q = queries @ Wq.T + bq
    k = keys @ Wk.T + bk
    v = values @ Wv.T + bv

    def split(x):
        b, l, e = x.shape
        return x.reshape(b, l, H, HD).transpose(0, 2, 1, 3)

    q, k, v = split(q), split(k), split(v)
    scores = np.einsum('bhtd,bhsd->bhts', q, k) / np.sqrt(HD)
    scores = np.where(key_padding_mask[:, None, None, :], -np.inf, scores)
    scores = np.where(~attn_mask[None, None, :, :], -np.inf, scores)
    scores = scores - scores.max(axis=-1, keepdims=True)
    w = np.exp(scores)
    w = w / w.sum(axis=-1, keepdims=True)
    w = np.where(q_padding_mask[:, None, :, None], 0.0, w)
    ctx = np.einsum('bhts,bhsd->bhtd', w, v)
    ctx = ctx.transpose(0, 2, 1, 3).reshape(queries.shape[0], -1, E)
    return (ctx @ Wo.T + bo).astype(np.float32)


def kernel(queries, keys, values, Wq, bq, Wk, bk, Wv, bv, Wo, bo,
           q_padding_mask, key_padding_mask, attn_mask):
    queries = np.asarray(queries, dtype=np.float32)
    keys = np.asarray(keys, dtype=np.float32)
    values = np.asarray(values, dtype=np.float32)
    Wq, bq = np.asarray(Wq, np.float32), np.asarray(bq, np.float32)
    Wk, bk = np.asarray(Wk, np.float32), np.asarray(bk, np.float32)
    Wv, bv = np.asarray(Wv, np.float32), np.asarray(bv, np.float32)
    Wo, bo = np.asarray(Wo, np.float32), np.asarray(bo, np.float32)
    q_padding_mask = np.asarray(q_padding_mask)
    key_padding_mask = np.asarray(key_padding_mask)
    attn_mask = np.asarray(attn_mask)

    # The device kernel skips masking (and softmax max-subtraction, valid for
    # this problem's bounded score range). Masks are all-trivial per the
    # problem spec; fall back to a host reference if they ever are not.
    if q_padding_mask.any() or key_padding_mask.any() or not attn_mask.all():
        return _numpy_reference(
            queries, keys, values, Wq, bq, Wk, bk, Wv, bv, Wo, bo,
            q_padding_mask, key_padding_mask, attn_mask)

    in_maps = make_in_maps(queries, keys, values, Wq, bq, Wk, bk, Wv, bv, Wo)
    res = run_device(in_maps, trace=False)
    out = np.empty((B, T, E), np.float32)
    for b in range(B):
        out[b] = (res.results[2 * b]["out"] + res.results[2 * b + 1]["out"]
                  + bo[None, :])
    return out
